# revision 5
# baseline (speedup 1.0000x reference)
"""Adaptive Computation Time (ACT) kernel for Trainium2, 8 NeuronCores.

Problem: hidden_states [B=8, T=12, S=4096, D=512] f32, halting head W_h [512],
b_h [1].  Per (b, s) position the reference runs a sequential halting scan over
the T step axis.  Reformulation used here: with q_t = sigmoid(h_t . W + b) and
C_t = cumsum_t(q) (C_0 = 0), the scan is branchless:

    rem_t      = [C_{t-1} < 0.99] * ( [C_t >= 0.99] ? (1 - C_{t-1}) : q_t )
    out        = sum_t rem_t * h_t
    n_updates  = sum_t [C_{t-1} < 0.99]
    cum_final  = sum_t q_t * [C_{t-1} < 0.99]
    running    = [C_T < 0.99]

Sharding: sequence-parallel along S (512 positions per core); W_h/b_h
replicated.  Each core processes 32 tiles of 128 positions:
  - DVE: fused dot (tensor_tensor_reduce) for logits, prefix-scan cumsum,
    small mask algebra.
  - ACT: sigmoid, diag(rem_t) builds, PSUM->SBUF eviction.
  - PE : out accumulated as sum_t diag(rem_t) @ h_t in PSUM (fp32r, 1 cyc/row).
  - The h tile is loaded from HBM exactly once (memory roofline).

The (almost-never-taken) global "never halted" branch and the tiny
ponder_cost / effective_steps tail are applied on the host, exactly following
the reference math.
"""

import os
import sys
from contextlib import ExitStack

import numpy as np

for _p in ("/root/.axon_site", "/root/.axon_site/_ro/trn_rl_repo",
           "/root/.axon_site/_ro/pypackages"):
    if os.path.isdir(_p) and _p not in sys.path:
        sys.path.append(_p)

import concourse.bacc as bacc
import concourse.mybir as mybir
import concourse.tile as tile
from concourse.bass_interp import get_hw_module
from concourse.bass_utils import run_bass_kernel_spmd

F32 = mybir.dt.float32
F32R = mybir.dt.float32r
AOP = mybir.AluOpType
ACTF = mybir.ActivationFunctionType
AXX = mybir.AxisListType.X

B, T, S, D = 8, 12, 4096, 512
NCORES = 8
S_SH = S // NCORES          # 512 positions per core (along S)
PT = 128                    # positions per tile (partition dim)
NSC = S_SH // PT            # 4 s-chunks per batch row
NT = B * NSC                # 32 tiles per core
THRESH = 0.99

_MODULE_CACHE = None


def _build_module():
    nc = bacc.Bacc("TRN2", target_bir_lowering=False, debug=False,
                   num_devices=NCORES)

    h = nc.dram_tensor("h", [B, T, S_SH, D], F32, kind="ExternalInput").ap()
    wb = nc.dram_tensor("wb", [PT, D], F32, kind="ExternalInput").ap()
    bb = nc.dram_tensor("bb", [PT, 1], F32, kind="ExternalInput").ap()
    iden = nc.dram_tensor("iden", [PT, PT], F32, kind="ExternalInput").ap()
    out = nc.dram_tensor("out", [B, S_SH, D], F32, kind="ExternalOutput").ap()
    stats = nc.dram_tensor("stats", [NT, PT, 3], F32, kind="ExternalOutput").ap()

    with tile.TileContext(nc) as tc, ExitStack() as ctx:
        cpool = ctx.enter_context(tc.tile_pool(name="consts", bufs=1))
        hpool = ctx.enter_context(tc.tile_pool(name="hbuf", bufs=2))
        spool = ctx.enter_context(tc.tile_pool(name="smalls", bufs=3))
        dpool = ctx.enter_context(tc.tile_pool(name="diag", bufs=2))
        opool = ctx.enter_context(tc.tile_pool(name="obuf", bufs=2))
        ppool = ctx.enter_context(tc.tile_pool(name="psum", bufs=2, space="PSUM"))

        wsb = cpool.tile([PT, D], F32)
        nc.sync.dma_start(out=wsb[:, :], in_=wb[:, :])
        bsb = cpool.tile([PT, 1], F32)
        nc.sync.dma_start(out=bsb[:, :], in_=bb[:, :])
        idsb = cpool.tile([PT, PT], F32)
        nc.sync.dma_start(out=idsb[:, :], in_=iden[:, :])
        junk = cpool.tile([PT, D], F32)

        for i in range(NT):
            b, sc = divmod(i, NSC)
            ht = hpool.tile([PT, T, D], F32, tag="ht")
            src = h[b, :, sc * PT:(sc + 1) * PT, :].rearrange("t p d -> p t d")
            nc.sync.dma_start(out=ht[:, :, :], in_=src)
            # fp32r copy for the PE moving operand (gpsimd casting DMA,
            # SBUF->SBUF: no HBM traffic, no compute-engine time)
            htr = hpool.tile([PT, T, D], F32R, tag="htr")
            nc.gpsimd.dma_start(out=htr[:, :, :], in_=ht[:, :, :])

            # logits L[:, t] = sum_d h_t * W  (fused multiply + free-dim
            # accumulate; stt is a standard ISA op — tensor_tensor_reduce is
            # custom-DVE ucode and faults at runtime on this NRT)
            L = spool.tile([PT, T], F32, tag="L")
            for t in range(T):
                nc.vector.scalar_tensor_tensor(
                    out=junk[:, :], in0=ht[:, t, :], scalar=0.0,
                    in1=wsb[:, :], op0=AOP.bypass, op1=AOP.mult,
                    accum_out=L[:, t:t + 1])

            # q = sigmoid(L + b)
            q = spool.tile([PT, T], F32, tag="q")
            nc.scalar.activation(q[:, :], L[:, :], ACTF.Sigmoid,
                                 bias=bsb[:, 0:1], scale=1.0)

            # C = cumsum_t(q), with C_0 = 0 prepended
            Cf = spool.tile([PT, T + 1], F32, tag="Cf")
            nc.vector.memset(Cf[:, 0:1], 0.0)
            nc.vector.tensor_tensor_scan(
                out=Cf[:, 1:T + 1], data0=q[:, :], data1=q[:, :],
                initial=0.0, op0=AOP.add, op1=AOP.bypass)
            Cp = Cf[:, 0:T]
            Cn = Cf[:, 1:T + 1]

            m1 = spool.tile([PT, T], F32, tag="m1")   # still running at step t
            nc.vector.tensor_scalar(out=m1[:, :], in0=Cp, scalar1=THRESH,
                                    scalar2=None, op0=AOP.is_lt)
            g = spool.tile([PT, T], mybir.dt.uint8, tag="g")  # crossed at t
            nc.vector.tensor_scalar(out=g[:, :], in0=Cn, scalar1=THRESH,
                                    scalar2=None, op0=AOP.is_ge)
            a1 = spool.tile([PT, T], F32, tag="a1")   # 1 - C_{t-1}
            nc.vector.tensor_scalar(out=a1[:, :], in0=Cp, scalar1=-1.0,
                                    scalar2=1.0, op0=AOP.mult, op1=AOP.add)
            sel = spool.tile([PT, T], F32, tag="sel")
            nc.vector.tensor_copy(sel[:, :], q[:, :])
            nc.vector.copy_predicated(sel[:, :], g[:, :], a1[:, :])
            rem = spool.tile([PT, T], F32, tag="rem")
            nc.vector.tensor_mul(rem[:, :], m1[:, :], sel[:, :])

            qm = spool.tile([PT, T], F32, tag="qm")
            nc.vector.tensor_mul(qm[:, :], q[:, :], m1[:, :])
            st = spool.tile([PT, 3], F32, tag="st")
            nc.vector.reduce_sum(out=st[:, 0:1], in_=qm[:, :], axis=AXX)  # cum
            nc.vector.reduce_sum(out=st[:, 1:2], in_=m1[:, :], axis=AXX)  # n_up
            nc.vector.tensor_scalar(out=st[:, 2:3], in0=Cn[:, T - 1:T],
                                    scalar1=THRESH, scalar2=None,
                                    op0=AOP.is_lt)                        # running
            nc.sync.dma_start(out=stats[i, :, :], in_=st[:, :])

            # out_tile = sum_t diag(rem_t) @ h_t   (PSUM accumulation)
            dg = dpool.tile([PT, T, PT], F32R, tag="dg")
            for t in range(T):
                nc.scalar.mul(dg[:, t, :], idsb[:, :], rem[:, t:t + 1])
            acc = ppool.tile([PT, D], F32, tag="acc")
            for t in range(T):
                nc.tensor.matmul(acc[:, :],
                                 dg[:, t, :],
                                 htr[:, t, :],
                                 start=(t == 0), stop=(t == T - 1))
            ot = opool.tile([PT, D], F32, tag="ot")
            nc.scalar.copy(ot[:, :], acc[:, :])
            nc.sync.dma_start(out=out[b, sc * PT:(sc + 1) * PT, :], in_=ot[:, :])

    nc.compile()
    return nc


def _get_module():
    global _MODULE_CACHE
    if _MODULE_CACHE is None:
        _MODULE_CACHE = _build_module()
    return _MODULE_CACHE


def _make_in_maps(hidden_states, W_h, b_h):
    wb = np.ascontiguousarray(np.broadcast_to(W_h[None, :], (PT, D)),
                              dtype=np.float32)
    bb = np.full((PT, 1), np.float32(b_h[0]), dtype=np.float32)
    iden = np.eye(PT, dtype=np.float32)
    in_maps = []
    for k in range(NCORES):
        shard = np.ascontiguousarray(
            hidden_states[:, :, k * S_SH:(k + 1) * S_SH, :], dtype=np.float32)
        in_maps.append({"h": shard, "wb": wb, "bb": bb, "iden": iden})
    return in_maps


def kernel(hidden_states, W_h, b_h):
    hidden_states = np.asarray(hidden_states, dtype=np.float32)
    W_h = np.asarray(W_h, dtype=np.float32)
    b_h = np.asarray(b_h, dtype=np.float32)
    assert hidden_states.shape == (B, T, S, D)

    nc = _get_module()
    in_maps = _make_in_maps(hidden_states, W_h, b_h)

    old_m = nc.m
    nc.m = get_hw_module(nc.m)
    try:
        res = run_bass_kernel_spmd(nc, in_maps, core_ids=list(range(NCORES)))
    finally:
        nc.m = old_m

    out_full = np.empty((B, S, D), np.float32)
    cum = np.empty((B, S), np.float32)
    nup = np.empty((B, S), np.float32)
    running = np.empty((B, S), np.float32)
    for k in range(NCORES):
        r = res.results[k]
        sl = slice(k * S_SH, (k + 1) * S_SH)
        out_full[:, sl, :] = r["out"]
        stt = r["stats"].reshape(B, NSC, PT, 3).reshape(B, S_SH, 3)
        cum[:, sl] = stt[:, :, 0]
        nup[:, sl] = stt[:, :, 1]
        running[:, sl] = stt[:, :, 2]

    # Host tail: exactly the reference's never-halted branch + ponder cost.
    if np.any(running > 0):
        h_last = hidden_states[:, -1, :, :]
        out_full = out_full + (1.0 - cum)[..., None] * h_last
        nup = nup + running
    eff = nup + np.clip(1.0 - cum, 0.0, None)
    ponder = eff.mean(axis=1, dtype=np.float32)
    return out_full, ponder.astype(np.float32), eff.astype(np.float32)


# revision 13
# speedup vs baseline: 60.3628x; 60.3628x over previous
"""Adaptive Computation Time (ACT) kernel for Trainium2, 8 NeuronCores.

Problem: hidden_states [B=8, T=12, S=4096, D=512] f32, halting head W_h [512],
b_h [1].  Per (b, s) position the reference runs a sequential halting scan over
the T step axis.  Reformulation used here: with q_t = sigmoid(h_t . W + b) and
C_t = cumsum_t(q) (C_0 = 0), the scan is branchless:

    rem_t      = [C_{t-1} < 0.99] * ( [C_t >= 0.99] ? (1 - C_{t-1}) : q_t )
    out        = sum_t rem_t * h_t
    n_updates  = sum_t [C_{t-1} < 0.99]
    cum_final  = sum_t q_t * [C_{t-1} < 0.99]
    running    = [C_T < 0.99]

Sharding: sequence-parallel along S (512 positions per core); W_h/b_h
replicated.  Each core processes 32 tiles of 128 positions:
  - DVE: fused dot (tensor_tensor_reduce) for logits, prefix-scan cumsum,
    small mask algebra.
  - ACT: sigmoid, diag(rem_t) builds, PSUM->SBUF eviction.
  - PE : out accumulated as sum_t diag(rem_t) @ h_t in PSUM (fp32r, 1 cyc/row).
  - The h tile is loaded from HBM exactly once (memory roofline).

The (almost-never-taken) global "never halted" branch and the tiny
ponder_cost / effective_steps tail are applied on the host, exactly following
the reference math.
"""

import os
import sys
from contextlib import ExitStack

import numpy as np

for _p in ("/root/.axon_site", "/root/.axon_site/_ro/trn_rl_repo",
           "/root/.axon_site/_ro/pypackages"):
    if os.path.isdir(_p) and _p not in sys.path:
        sys.path.append(_p)

import concourse.bacc as bacc
import concourse.mybir as mybir
import concourse.tile as tile
from concourse.bass_interp import get_hw_module
from concourse.bass_utils import run_bass_kernel_spmd

F32 = mybir.dt.float32
F32R = mybir.dt.float32r
AOP = mybir.AluOpType
ACTF = mybir.ActivationFunctionType
AXX = mybir.AxisListType.X

B, T, S, D = 8, 12, 4096, 512
NCORES = 8
S_SH = S // NCORES          # 512 positions per core (along S)
PT = 128                    # positions per tile (partition dim)
NSC = S_SH // PT            # 4 s-chunks per batch row
NT = B * NSC                # 32 tiles per core
THRESH = 0.99

_MODULE_CACHE = None


def _build_module(repeat=1):
    nc = bacc.Bacc("TRN2", target_bir_lowering=False, debug=False,
                   num_devices=NCORES)

    h = nc.dram_tensor("h", [B, T, S_SH, D], F32, kind="ExternalInput").ap()
    wb = nc.dram_tensor("wb", [PT, D], F32, kind="ExternalInput").ap()
    bb = nc.dram_tensor("bb", [PT, 1], F32, kind="ExternalInput").ap()
    iden = nc.dram_tensor("iden", [PT, PT], F32, kind="ExternalInput").ap()
    out = nc.dram_tensor("out", [B, S_SH, D], F32, kind="ExternalOutput").ap()
    stats = nc.dram_tensor("stats", [PT, NT * 3], F32, kind="ExternalOutput").ap()

    with tile.TileContext(nc) as tc, ExitStack() as ctx:
        cpool = ctx.enter_context(tc.tile_pool(name="consts", bufs=1))
        hpool = ctx.enter_context(tc.tile_pool(name="hbuf", bufs=2))
        spool = ctx.enter_context(tc.tile_pool(name="smalls", bufs=3))
        dpool = ctx.enter_context(tc.tile_pool(name="diag", bufs=2))
        opool = ctx.enter_context(tc.tile_pool(name="obuf", bufs=2))
        ppool = ctx.enter_context(tc.tile_pool(name="psum", bufs=2, space="PSUM"))

        wsb = cpool.tile([PT, D], F32)
        nc.sync.dma_start(out=wsb[:, :], in_=wb[:, :])
        bsb = cpool.tile([PT, 1], F32)
        nc.sync.dma_start(out=bsb[:, :], in_=bb[:, :])
        idsb = cpool.tile([PT, PT], F32)
        nc.sync.dma_start(out=idsb[:, :], in_=iden[:, :])
        junk = cpool.tile([PT, D], F32)
        stats_all = cpool.tile([PT, NT * 3], F32)

        rep_cm = tc.For_i(0, repeat, 1) if repeat > 1 else None
        if rep_cm is not None:
            ctx.enter_context(rep_cm)
        for i in range(NT):
            b, sc = divmod(i, NSC)
            ht = hpool.tile([PT, T, D], F32, tag="ht")
            src = h[b, :, sc * PT:(sc + 1) * PT, :].rearrange("t p d -> p t d")
            nc.sync.dma_start(out=ht[:, :, :], in_=src)
            # fp32r copy for the PE moving operand, on the otherwise-idle
            # GPSIMD engine (a casting DMA would contend for the shared
            # per-core DMA fabric that the HBM loads already saturate)
            htr = hpool.tile([PT, T, D], F32R, tag="htr")
            nc.gpsimd.tensor_copy(htr[:, :, :], ht[:, :, :])

            # logits L[:, t] = sum_d h_t * W  (fused multiply + free-dim
            # accumulate; stt is a standard ISA op — tensor_tensor_reduce is
            # custom-DVE ucode and faults at runtime on this NRT)
            L = spool.tile([PT, T], F32, tag="L")
            for t in range(T):
                nc.vector.scalar_tensor_tensor(
                    out=junk[:, :], in0=ht[:, t, :], scalar=0.0,
                    in1=wsb[:, :], op0=AOP.bypass, op1=AOP.mult,
                    accum_out=L[:, t:t + 1])

            # q = sigmoid(L + b)
            q = spool.tile([PT, T], F32, tag="q")
            nc.scalar.activation(q[:, :], L[:, :], ACTF.Sigmoid,
                                 bias=bsb[:, 0:1], scale=1.0)

            # C = cumsum_t(q), with C_0 = 0 prepended
            Cf = spool.tile([PT, T + 1], F32, tag="Cf")
            nc.vector.memset(Cf[:, 0:1], 0.0)
            nc.vector.tensor_tensor_scan(
                out=Cf[:, 1:T + 1], data0=q[:, :], data1=q[:, :],
                initial=0.0, op0=AOP.add, op1=AOP.bypass)
            Cp = Cf[:, 0:T]
            Cn = Cf[:, 1:T + 1]

            m1 = spool.tile([PT, T], F32, tag="m1")   # still running at step t
            nc.vector.tensor_scalar(out=m1[:, :], in0=Cp, scalar1=THRESH,
                                    scalar2=None, op0=AOP.is_lt)
            g = spool.tile([PT, T], mybir.dt.uint8, tag="g")  # crossed at t
            nc.vector.tensor_scalar(out=g[:, :], in0=Cn, scalar1=THRESH,
                                    scalar2=None, op0=AOP.is_ge)
            a1 = spool.tile([PT, T], F32, tag="a1")   # 1 - C_{t-1}
            nc.vector.tensor_scalar(out=a1[:, :], in0=Cp, scalar1=-1.0,
                                    scalar2=1.0, op0=AOP.mult, op1=AOP.add)
            sel = spool.tile([PT, T], F32, tag="sel")
            nc.vector.tensor_copy(sel[:, :], q[:, :])
            nc.vector.copy_predicated(sel[:, :], g[:, :], a1[:, :])
            rem = spool.tile([PT, T], F32, tag="rem")
            nc.vector.tensor_mul(rem[:, :], m1[:, :], sel[:, :])

            qm = spool.tile([PT, T], F32, tag="qm")
            nc.vector.tensor_mul(qm[:, :], q[:, :], m1[:, :])
            st = stats_all[:, 3 * i:3 * i + 3]
            nc.vector.reduce_sum(out=st[:, 0:1], in_=qm[:, :], axis=AXX)  # cum
            nc.vector.reduce_sum(out=st[:, 1:2], in_=m1[:, :], axis=AXX)  # n_up
            nc.vector.tensor_scalar(out=st[:, 2:3], in0=Cn[:, T - 1:T],
                                    scalar1=THRESH, scalar2=None,
                                    op0=AOP.is_lt)                        # running

            # out_tile = sum_t diag(rem_t) @ h_t   (PSUM accumulation)
            dg = dpool.tile([PT, T, PT], F32R, tag="dg")
            for t in range(T):
                nc.scalar.mul(dg[:, t, :], idsb[:, :], rem[:, t:t + 1])
            acc = ppool.tile([PT, D], F32, tag="acc")
            for t in range(T):
                nc.tensor.matmul(acc[:, :],
                                 dg[:, t, :],
                                 htr[:, t, :],
                                 start=(t == 0), stop=(t == T - 1))
            ot = opool.tile([PT, D], F32, tag="ot")
            nc.scalar.copy(ot[:, :], acc[:, :])
            nc.sync.dma_start(out=out[b, sc * PT:(sc + 1) * PT, :], in_=ot[:, :])

        nc.sync.dma_start(out=stats[:, :], in_=stats_all[:, :])

    nc.compile()
    return nc


def _get_module():
    global _MODULE_CACHE
    if _MODULE_CACHE is None:
        _MODULE_CACHE = _build_module()
    return _MODULE_CACHE


def _make_in_maps(hidden_states, W_h, b_h):
    wb = np.ascontiguousarray(np.broadcast_to(W_h[None, :], (PT, D)),
                              dtype=np.float32)
    bb = np.full((PT, 1), np.float32(b_h[0]), dtype=np.float32)
    iden = np.eye(PT, dtype=np.float32)
    in_maps = []
    for k in range(NCORES):
        shard = np.ascontiguousarray(
            hidden_states[:, :, k * S_SH:(k + 1) * S_SH, :], dtype=np.float32)
        in_maps.append({"h": shard, "wb": wb, "bb": bb, "iden": iden})
    return in_maps


def kernel(hidden_states, W_h, b_h):
    hidden_states = np.asarray(hidden_states, dtype=np.float32)
    W_h = np.asarray(W_h, dtype=np.float32)
    b_h = np.asarray(b_h, dtype=np.float32)
    assert hidden_states.shape == (B, T, S, D)

    nc = _get_module()
    in_maps = _make_in_maps(hidden_states, W_h, b_h)

    old_m = nc.m
    nc.m = get_hw_module(nc.m)
    try:
        res = run_bass_kernel_spmd(nc, in_maps, core_ids=list(range(NCORES)))
    finally:
        nc.m = old_m

    out_full = np.empty((B, S, D), np.float32)
    cum = np.empty((B, S), np.float32)
    nup = np.empty((B, S), np.float32)
    running = np.empty((B, S), np.float32)
    for k in range(NCORES):
        r = res.results[k]
        sl = slice(k * S_SH, (k + 1) * S_SH)
        out_full[:, sl, :] = r["out"]
        stt = (r["stats"].reshape(PT, NT, 3).transpose(1, 0, 2)
               .reshape(B, NSC, PT, 3).reshape(B, S_SH, 3))
        cum[:, sl] = stt[:, :, 0]
        nup[:, sl] = stt[:, :, 1]
        running[:, sl] = stt[:, :, 2]

    # Host tail: exactly the reference's never-halted branch + ponder cost.
    if np.any(running > 0):
        h_last = hidden_states[:, -1, :, :]
        out_full = out_full + (1.0 - cum)[..., None] * h_last
        nup = nup + running
    eff = nup + np.clip(1.0 - cum, 0.0, None)
    ponder = eff.mean(axis=1, dtype=np.float32)
    return out_full, ponder.astype(np.float32), eff.astype(np.float32)
